# revision 3
# baseline (speedup 1.0000x reference)
"""Continual-attention Trainium2 kernel (8 NeuronCores, SPMD).

Sharding: core c -> batch b = c//2, head-group g = c%2 (4 heads each).
Per (b,h) computes S^T[k,q] = K Q^T via PE (fp16, zero-padded to 128
contraction rows), then per k-tile-triple [128,1536] PSUM group either:
  - ACT: exp with fused 1/sqrt(d) scale -> pt fp16 (+ DVE 0/1 diagonal /
    chunk mask multiplies on the 128-col diag sub-blocks), or
  - DVE: fused Schraudolph exp + per-batch test-train mask in ONE
    scalar_tensor_tensor: y_i16 = round(S*A0 + mttb), where mttb carries
    B0 (allowed) or B0-50000 (masked -> int16 saturates -> bitcast -0.0).
    pt is the int16 tile bitcast to fp16 (piecewise-linear exp, ~1.8% rms).
Then O^T[d,q] (+ softmax denominator as a 65th row via a ones column in V)
accumulated on PE from pt. Normalization + final transpose happen on host.
"""

import sys

sys.path.insert(0, "/opt/trn_rl_repo")

import numpy as np

B, L, H, D = 4, 2048, 8, 64
TRAIN = 1536
TEST = L - TRAIN            # 512
NCH = 64                    # test chunks
CH = TEST // NCH            # 8
HPC = 4                     # heads per core
NCORES = 8
KT = L // 128               # 16 k-tiles

# Schraudolph fp16-bitcast exp constants (applied to RAW logits; the 1/8
# scale is folded into A0).  sigma=60 centers the piecewise-linear error.
A0 = float((2.0**10) / np.log(2.0) * 0.125)
B0 = 15.0 * 1024 - 60.0
BMASK = B0 - 50000.0        # forces int16 saturation -> fp16 -0.0

LAST_RESULT = None          # BassKernelResults of the most recent run
_PROG = None                # cached compiled Bass program


def _split_multi_waits(nc, mybir):
    """This container's walrus accepts at most one semaphore wait per
    instruction; Tile's tail drains can carry several. Hoist extras onto
    NoOps inserted immediately before, on the same engine."""
    for f in nc.m.functions:
        for bb in f.blocks:
            insts = list(bb.instructions)
            out = []
            changed = False
            for inst in insts:
                si = inst.sync_info
                if si is not None and len(si.on_wait) > 1:
                    waits = list(si.on_wait)
                    for w in waits[:-1]:
                        nop = mybir.InstNoOp(
                            name=f"waitnop-{nc.next_id()}", ins=[], outs=[]
                        )
                        nop.engine = inst.engine
                        nop.sync_info = mybir.SyncInfo(on_wait=[w], on_update=[])
                        out.append(nop)
                    inst.sync_info = mybir.SyncInfo(
                        on_wait=[waits[-1]], on_update=list(si.on_update)
                    )
                    changed = True
                out.append(inst)
            if changed:
                bb.instructions = out


def _groups_for_gq(gq):
    """k-tile groups per q-group: list of (kps, engine) where engine is
    'act' or 'dve' (dve = fused schraudolph+mtt, only gq3 train tiles)."""
    if gq == 0:
        return [([0, 1, 2], "act"), ([3], "act")]
    if gq == 1:
        return [([0, 1, 2], "act"), ([3, 4, 5], "act"), ([6, 7], "act")]
    if gq == 2:
        return [
            ([0, 1, 2], "act"),
            ([3, 4, 5], "act"),
            ([6, 7, 8], "act"),
            ([9, 10, 11], "act"),
        ]
    return [
        ([0, 1, 2], "dve"),
        ([3, 4, 5], "dve"),
        ([6, 7, 8], "dve"),
        ([9, 10, 11], "dve"),
        ([12, 13, 14], "act"),
        ([15], "act"),
    ]


def _geom(gq, kp):
    """(off, w): query-column offset within the 512-wide q-group and width
    of the causally-needed slice for k-tile kp."""
    if kp <= 11:
        off = max(0, 128 * kp - 512 * gq)
        return off, 512 - off
    off = 128 * (kp - 12)
    return off, 128


def _build_program():
    import concourse.bass as bass
    import concourse.mybir as mybir
    import concourse.tile as tile

    f32 = mybir.dt.float32
    fp16 = mybir.dt.float16
    i16 = mybir.dt.int16
    Exp = mybir.ActivationFunctionType.Exp
    Alu = mybir.AluOpType

    nc = bass.Bass()

    qt_d = nc.dram_tensor("qt", [HPC, 128, L], fp16, kind="ExternalInput")
    kt_d = nc.dram_tensor("kt", [HPC, 128, L], fp16, kind="ExternalInput")
    vw_d = nc.dram_tensor("vw", [HPC, 128, KT * 65], fp16, kind="ExternalInput")
    mttb_d = nc.dram_tensor("mttb", [128, 12 * 512], fp16, kind="ExternalInput")
    mdiag_d = nc.dram_tensor("mdiag", [128, 128], fp16, kind="ExternalInput")
    mchunk_d = nc.dram_tensor("mchunk", [128, 128], fp16, kind="ExternalInput")
    ot_d = nc.dram_tensor("ot", [HPC, 65, L], f32, kind="ExternalOutput")

    with tile.TileContext(nc) as tc:
        with (
            tc.tile_pool(name="consts", bufs=1) as consts,
            tc.tile_pool(name="heads", bufs=2) as heads,
            tc.tile_pool(name="ptp", bufs=6) as ptp,
            tc.tile_pool(name="osbp", bufs=3) as osbp,
            tc.tile_pool(name="spp", bufs=2, space="PSUM") as spp,
            tc.tile_pool(name="avp", bufs=2, space="PSUM") as avp,
        ):
            mdiag_sb = consts.tile([128, 128], fp16)
            nc.sync.dma_start(out=mdiag_sb, in_=mdiag_d.ap())
            mchunk_sb = consts.tile([128, 128], fp16)
            nc.sync.dma_start(out=mchunk_sb, in_=mchunk_d.ap())
            mttb_sb = consts.tile([128, 12 * 512], fp16)

            first = True
            for h in range(HPC):
                # qt/kt are zero-padded to 128 contraction rows: K=64 matmuls
                # never leave the PE's throttled clock state (HW-measured
                # 430ns vs 216ns per 512-col matmul), K=128 ones do.
                # DMAs are split into chunks so head-0 compute can start as
                # soon as its first k/q columns land.
                qt_sb = heads.tile([128, L], fp16, tag="qt")
                kt_sb = heads.tile([128, L], fp16, tag="kt")
                vw_sb = heads.tile([128, KT, 65], fp16, tag="vw")
                nch = 4 if first else 1
                step = L // nch
                for i in range(nch):
                    sl = slice(i * step, (i + 1) * step)
                    nc.sync.dma_start(out=kt_sb[:, sl], in_=kt_d.ap()[h][:, sl])
                for i in range(nch):
                    sl = slice(i * step, (i + 1) * step)
                    nc.sync.dma_start(out=qt_sb[:, sl], in_=qt_d.ap()[h][:, sl])
                nc.sync.dma_start(
                    out=vw_sb,
                    in_=vw_d.ap()[h].rearrange("p (t c) -> p t c", t=KT),
                )
                if first:
                    nc.sync.dma_start(out=mttb_sb, in_=mttb_d.ap())
                    first = False

                for gq in range(4):
                    av = avp.tile([128, 512], f32, tag="av")
                    groups = _groups_for_gq(gq)
                    last_kp = groups[-1][0][-1]

                    PIPE = 2  # groups of AV matmuls held back
                    pending = []

                    def emit_av(kp, pt, pos, off, w, start, stop):
                        nc.tensor.matmul(
                            av[:65, off : off + w],
                            lhsT=vw_sb[:, kp, :],
                            rhs=pt[:, pos : pos + w],
                            start=start,
                            stop=stop,
                            skip_group_check=True,
                        )

                    for kps, eng in groups:
                        # pack tiles into the group PSUM tile without any
                        # matmul output crossing a 512-col (2KB) PSUM bank
                        # boundary: first-fit into 512-col banks
                        geo = [_geom(gq, kp) for kp in kps]
                        poss = []
                        cur = 0
                        for _, w in geo:
                            if (cur % 512) + w > 512:
                                cur = (cur // 512 + 1) * 512
                            poss.append(cur)
                            cur += w
                        span = cur

                        sp = spp.tile([128, 1536], f32, tag="sp")
                        for kp, (off, w), pos in zip(kps, geo, poss):
                            qs = 512 * gq + off
                            nc.tensor.matmul(
                                sp[:, pos : pos + w],
                                lhsT=kt_sb[:, 128 * kp : 128 * kp + 128],
                                rhs=qt_sb[:, qs : qs + w],
                                start=True,
                                stop=True,
                                skip_group_check=True,
                            )

                        pt = ptp.tile([128, 1536], fp16, tag="pt")
                        if eng == "dve":
                            # fused schraudolph exp + per-batch test-train
                            # mask; kps are 512-aligned train tiles so the
                            # mttb slice is [512*kps[0], 512*(kps[-1]+1])
                            ms = 512 * kps[0]
                            nc.vector.scalar_tensor_tensor(
                                pt.bitcast(i16)[:, 0:span],
                                sp[:, 0:span],
                                A0,
                                mttb_sb[:, ms : ms + span],
                                op0=Alu.mult,
                                op1=Alu.add,
                            )
                        else:
                            nc.scalar.activation(
                                pt[:, 0:span], sp[:, 0:span], Exp, scale=0.125
                            )
                            for kp, pos in zip(kps, poss):
                                if kp <= 11 and 128 * kp >= 512 * gq:
                                    nc.vector.tensor_mul(
                                        pt[:, pos : pos + 128],
                                        pt[:, pos : pos + 128],
                                        mdiag_sb,
                                    )
                                elif kp >= 12:
                                    nc.vector.tensor_mul(
                                        pt[:, pos : pos + 128],
                                        pt[:, pos : pos + 128],
                                        mchunk_sb,
                                    )

                        pending.append(
                            [
                                (kp, pt, pos, off, w, kp == 0, kp == last_kp)
                                for kp, (off, w), pos in zip(kps, geo, poss)
                            ]
                        )
                        if len(pending) > PIPE:
                            for args in pending.pop(0):
                                emit_av(*args)

                    for grp in pending:
                        for args in grp:
                            emit_av(*args)

                    osb = osbp.tile([65, 512], f32)
                    nc.vector.tensor_copy(osb, av[:65, :])
                    nc.sync.dma_start(
                        out=ot_d.ap()[h][:, 512 * gq : 512 * gq + 512], in_=osb
                    )

    import concourse.mybir as mybir_mod

    _split_multi_waits(nc, mybir_mod)
    return nc


def _host_inputs(queries, keys, values, attach):
    """Build per-core input maps (host-side layout prep)."""
    f16 = np.float16
    p = np.arange(128)
    f = np.arange(128)
    mdiag = np.where(f[None, :] >= p[:, None], 1.0, 0.0).astype(np.float32)
    mchunk = np.where(
        (p[:, None] // CH == f[None, :] // CH) & (p[:, None] <= f[None, :]),
        1.0,
        0.0,
    ).astype(np.float32)

    in_maps = []
    for c in range(NCORES):
        b, g = divmod(c, 2)
        hs = slice(HPC * g, HPC * (g + 1))
        q = queries[b][:, hs, :]          # [L, 4, D]
        k = keys[b][:, hs, :]
        v = values[b][:, hs, :]
        qt = np.zeros((HPC, 128, L), np.float32)
        qt[:, :D, :] = q.transpose(1, 2, 0)
        kt = np.zeros((HPC, 128, L), np.float32)
        kt[:, :D, :] = k.transpose(1, 2, 0)
        vw = np.empty((HPC, L, 65), np.float32)
        vw[:, :, :64] = v.transpose(1, 0, 2)
        vw[:, :, 64] = 1.0
        # [4, L, 65] -> [4, 128, KT*65] with row p holding tile-chunks
        vw = np.ascontiguousarray(
            vw.reshape(HPC, KT, 128, 65).transpose(0, 2, 1, 3).reshape(HPC, 128, KT * 65)
        )
        kg = (np.arange(12)[:, None] * 128 + np.arange(128)[None, :])  # [12,128]
        thr = attach[b][np.arange(TEST) // CH]                          # [512]
        mttb = np.where(kg[:, :, None] <= thr[None, None, :], B0, BMASK)
        mttb = np.ascontiguousarray(
            mttb.transpose(1, 0, 2).reshape(128, 12 * 512)
        )
        in_maps.append(
            {
                "qt": qt.astype(f16),
                "kt": kt.astype(f16),
                "vw": vw.astype(f16),
                "mttb": mttb.astype(f16),
                "mdiag": mdiag.astype(f16),
                "mchunk": mchunk.astype(f16),
            }
        )
    return in_maps


def kernel(queries, keys, values, attach_test_after, train_len):
    global LAST_RESULT, _PROG
    import os

    queries = np.asarray(queries, dtype=np.float32)
    keys = np.asarray(keys, dtype=np.float32)
    values = np.asarray(values, dtype=np.float32)
    attach = np.asarray(attach_test_after).astype(np.int64)
    tl = int(np.asarray(train_len))
    assert queries.shape == (B, L, H, D), queries.shape
    assert tl == TRAIN and attach.shape == (B, NCH)

    from concourse.bass_utils import run_bass_kernel_spmd

    if _PROG is None:
        _PROG = _build_program()

    in_maps = _host_inputs(queries, keys, values, attach)
    trace = bool(int(os.environ.get("KERNEL_TRACE", "0")))
    res = run_bass_kernel_spmd(
        _PROG, in_maps, core_ids=list(range(NCORES)), trace=trace
    )
    LAST_RESULT = res

    out = np.empty((B, L, H * D), np.float32)
    for c in range(NCORES):
        b, g = divmod(c, 2)
        ot = res.results[c]["ot"]                     # [4, 65, L]
        o = ot[:, :64, :] / ot[:, 64:65, :]           # [4, 64, L]
        out[b, :, 256 * g : 256 * (g + 1)] = (
            o.transpose(2, 0, 1).reshape(L, HPC * D)
        )
    return out
